# revision 30
# baseline (speedup 1.0000x reference)
"""GCN layer kernel for 8 Trainium2 NeuronCores (Bass/Tile).

Math:  out = A @ (x @ W)  computed as  (A @ x) @ W
where A is the sparse [N, N] adjacency given as (rows, cols, vals) COO.

Sharding: destination nodes (output rows) are split into 8 contiguous
blocks of R = N/8, one per core (SPMD: identical program, per-core data).
Each core's edges are grouped on the host by 128-row destination
"window"; because dma_gather indices are int16, the source-node space is
split into Q=4 quadrants of 25000 rows, and every (window, quadrant)
segment gets a fixed budget of CPQ chunks x 128 edge slots.  Windows are
processed in groups of G; one dma_gather call per (group, quadrant)
fetches all 512-byte source rows x[col[e], :] for that segment batch.
Per 128-edge chunk the core builds a val-scaled one-hot
[128 edges, 128 window-rows] with one fused tensor_scalar
(is_equal then mult) and matmul-accumulates  psum[feat, wrow] += x_g^T @ onehot.
After a window's 20 chunks, a second matmul applies W:
  outT[:, window] = W^T @ psum  ->  [64, 128] -> DMA to DRAM.
Unused slots gather quadrant row 0 with local row -1 (one-hot row is
identically zero), contributing exactly zero.
"""

import sys
from contextlib import ExitStack

import numpy as np

sys.path.insert(0, "/opt/trn_rl_repo")

import concourse.bacc as bacc
import concourse.bass as bass
import concourse.mybir as mybir
import concourse.tile as tile
from concourse import library_config

N_NODES = 100000
N_EDGES = 1600000
F_IN = 128
F_OUT = 64
N_CORES = 8

R = N_NODES // N_CORES  # 12500 destination rows per core
WIN = -(-R // 128)  # 98 windows of 128 dest rows per core
Q = 4  # source quadrants (int16 idx limit)
QS = N_NODES // Q  # 25000 rows per quadrant
CPQ = 5  # chunk budget per (window, quadrant) segment: 640 edges (mean 512, sigma 23)
CPW = Q * CPQ  # 20 chunks per window
G = 7  # windows per gather group
NGRP = WIN // G  # 14 groups
NI = G * CPQ * 128  # 4480 idxs per dma_gather call
CPC = NI // 128  # 35 chunks per call
NCALL = NGRP * Q  # 56 calls
NSLOT_COL = NCALL * CPC  # 1960 chunk columns
IDX_COLS = NCALL * (NI // 16)  # 15680 int16 idx columns
PADW = WIN * 128  # 12544 padded output columns


def build_program(
    cpq=CPQ, repeat=1, oh_gps_every=0, oh_eng="any", gbufs=8, ohbufs=8, trim=False
):
    cpw = Q * cpq
    ni = G * cpq * 128
    cpc = ni // 128
    nslot_col = NGRP * Q * cpc
    idx_cols = NGRP * Q * (ni // 16)
    f32 = mybir.dt.float32
    i16 = mybir.dt.int16
    # Tile schedules with the cost model; its stock dma_gather estimate
    # (0.34 ns/descriptor) is ~27x below measured HW (~9.2 ns serial,
    # ~3 ns with 4 SWDGE queues), which starves gather prefetch in the
    # emitted order. Patch the spec so the scheduler plans realistically.
    from concourse.hw_specs import TRN2Spec

    TRN2Spec.SWDGE_NS_PER_DESCRIPTOR = 4.0
    nc = bacc.Bacc(
        "TRN2",
        target_bir_lowering=False,
        debug=False,
        enable_asserts=False,
        num_devices=N_CORES,
        num_swdge_queues=4,
    )
    x_d = nc.dram_tensor("xg", [N_NODES, F_IN], f32, kind="ExternalInput").ap()
    w_d = nc.dram_tensor("w", [F_IN, F_OUT], f32, kind="ExternalInput").ap()
    if trim:
        ncalls = NGRP * G * Q
        cnt_d = nc.dram_tensor(
            "cnt", [1, ncalls], mybir.dt.int32, kind="ExternalInput"
        ).ap()
    idx_d = nc.dram_tensor("idx", [128, idx_cols], i16, kind="ExternalInput").ap()
    lr_d = nc.dram_tensor("lr", [128, nslot_col], f32, kind="ExternalInput").ap()
    val_d = nc.dram_tensor("val", [128, nslot_col], f32, kind="ExternalInput").ap()
    iota_d = nc.dram_tensor("iota", [128, 128], f32, kind="ExternalInput").ap()
    out_d = nc.dram_tensor("outT", [F_OUT, PADW], f32, kind="ExternalOutput").ap()

    with tile.TileContext(nc) as tc, ExitStack() as ctx:
        const = ctx.enter_context(tc.tile_pool(name="const", bufs=1))
        idxp = ctx.enter_context(tc.tile_pool(name="idxp", bufs=2))
        gp = ctx.enter_context(tc.tile_pool(name="gp", bufs=gbufs))
        ohp = ctx.enter_context(tc.tile_pool(name="ohp", bufs=ohbufs))
        axp = ctx.enter_context(tc.tile_pool(name="axp", bufs=2))
        osp = ctx.enter_context(tc.tile_pool(name="osp", bufs=2))
        psa = ctx.enter_context(tc.tile_pool(name="psa", bufs=3, space="PSUM"))
        psb = ctx.enter_context(tc.tile_pool(name="psb", bufs=2, space="PSUM"))

        nc.gpsimd.load_library(library_config.mlp)

        w_sb = const.tile([F_IN, F_OUT], f32)
        iota_sb = const.tile([128, 128], f32)
        lr_sb = const.tile([128, nslot_col], f32)
        val_sb = const.tile([128, nslot_col], f32)
        nc.sync.dma_start(out=w_sb[:], in_=w_d[:])
        nc.sync.dma_start(out=iota_sb[:], in_=iota_d[:])
        nc.sync.dma_start(out=lr_sb[:], in_=lr_d[:])
        nc.sync.dma_start(out=val_sb[:], in_=val_d[:])

        if trim:
            cnt_sb = const.tile([1, NGRP * G * Q], mybir.dt.int32)
            nc.sync.dma_start(out=cnt_sb[:], in_=cnt_d[:])
            cnt_reg = nc.gpsimd.alloc_register("gcnt")
            # stale SBUF in trimmed (unwritten) gather slots must be finite
            for _ in range(gbufs):
                g0 = gp.tile([128, cpq, F_IN], f32, tag="g")
                nc.gpsimd.memset(g0[:], 0.0)

        icpg = Q * (ni // 16)  # idx cols per group
        icps = cpq * 128 // 16  # idx cols per (window, quadrant) segment
        for g in [g for _ in range(repeat) for g in range(NGRP)]:
            idx_t = idxp.tile([128, icpg], i16, tag="idx")
            nc.sync.dma_start(out=idx_t[:], in_=idx_d[:, g * icpg : (g + 1) * icpg])
            gts = []
            if not trim:
                for q in range(Q):
                    gt = gp.tile([128, cpc, F_IN], f32, tag="g")
                    nc.gpsimd.dma_gather(
                        gt[:],
                        x_d[q * QS : (q + 1) * QS, :],
                        idx_t[:, q * (ni // 16) : (q + 1) * (ni // 16)],
                        ni,
                        ni,
                        F_IN,
                        single_packet=False,
                        queue_num=q,
                    )
                    gts.append(gt)
            else:
                for wl in range(G):
                    for q in range(Q):
                        gt = gp.tile([128, cpq, F_IN], f32, tag="g")
                        base = q * (ni // 16) + wl * icps
                        call = (g % NGRP) * (G * Q) + wl * Q + q
                        # the decode-side ring accounting needs the exact
                        # post-trim index count; load it per call
                        with tc.tile_critical():
                            nc.gpsimd.reg_load(
                                cnt_reg, cnt_sb[0:1, call : call + 1]
                            )
                            nc.gpsimd.dma_gather(
                                gt[:],
                                x_d[q * QS : (q + 1) * QS, :],
                                idx_t[:, base : base + icps],
                                cpq * 128,
                                cnt_reg,
                                F_IN,
                                single_packet=False,
                                queue_num=q,
                            )
                        gts.append(gt)
            ost = osp.tile([F_OUT, G * 128], f32, tag="ost")
            for wl in range(G):
                pa = psa.tile([F_IN, 128], f32)
                k = 0
                for q in range(Q):
                    for j in range(cpq):
                        col = (g * Q + q) * cpc + wl * cpq + j
                        oh = ohp.tile([128, 128], f32)
                        if oh_gps_every and k % oh_gps_every == oh_gps_every - 1:
                            eng = nc.gpsimd
                        else:
                            eng = {"any": nc.any, "vector": nc.vector}[oh_eng]
                        eng.tensor_scalar(
                            out=oh[:],
                            in0=iota_sb[:],
                            scalar1=lr_sb[:, col : col + 1],
                            scalar2=val_sb[:, col : col + 1],
                            op0=mybir.AluOpType.is_equal,
                            op1=mybir.AluOpType.mult,
                        )
                        if trim:
                            lhsT = gts[wl * Q + q][:, j, :]
                        else:
                            lhsT = gts[q][:, wl * cpq + j, :]
                        nc.tensor.matmul(
                            out=pa[:],
                            lhsT=lhsT,
                            rhs=oh[:],
                            start=(k == 0),
                            stop=(k == cpw - 1),
                        )
                        k += 1
                axt = axp.tile([F_IN, 128], f32)
                nc.scalar.copy(out=axt[:], in_=pa[:])
                pb = psb.tile([F_OUT, 128], f32)
                nc.tensor.matmul(
                    out=pb[:], lhsT=w_sb[:], rhs=axt[:], start=True, stop=True
                )
                nc.scalar.copy(out=ost[:, wl * 128 : (wl + 1) * 128], in_=pb[:])
            nc.sync.dma_start(
                out=out_d[:, g * G * 128 : (g + 1) * G * 128], in_=ost[:]
            )

    nc.compile()
    return nc


def shard_inputs(x, weight, rows, cols, vals, cpq=CPQ, trim=False):
    """Group edges by (dest core, dest window, source quadrant) into the
    fixed per-call slot grids consumed by the device program."""
    cpw = Q * cpq
    ni = G * cpq * 128
    cpc = ni // 128
    nslot_col = NGRP * Q * cpc
    idx_cols = NGRP * Q * (ni // 16)
    seg_cap = cpq * 128

    core = rows // R
    loc = rows % R
    w = loc // 128
    lr = (loc % 128).astype(np.float32)
    quad = cols // QS
    qidx = (cols % QS).astype(np.int16)
    # segment id, ordered so positions within each segment can be assigned
    seg = ((core * NGRP + w // G) * Q + quad) * G + (w % G)
    n_seg = N_CORES * NGRP * Q * G
    order = np.argsort(seg, kind="stable")
    seg_s = seg[order]
    counts = np.bincount(seg_s, minlength=n_seg)
    if counts.max() > seg_cap:
        raise OverflowError(
            f"segment capacity exceeded: {counts.max()} > {seg_cap}"
        )
    starts = np.zeros(n_seg, np.int64)
    np.cumsum(counts[:-1], out=starts[1:])
    pos = np.arange(len(rows)) - starts[seg_s]

    core_s = seg_s // (NGRP * Q * G)
    g_s = (seg_s // (Q * G)) % NGRP
    q_s = (seg_s // G) % Q
    wl_s = seg_s % G
    call = g_s * Q + q_s
    i_call = wl_s * seg_cap + pos  # flat index within the call
    # idx array: element i -> (partition i%16, col i//16), replicated x8
    idx_all = np.full((N_CORES, 16, idx_cols), -1 if trim else 0, np.int16)
    idx_all[core_s, i_call % 16, call * (ni // 16) + i_call // 16] = qidx[order]
    idx_all = np.tile(idx_all, (1, 8, 1))
    # chunk arrays: slot (p, chunk col)
    p_s = i_call % 128
    col_s = call * cpc + i_call // 128
    lr_all = np.full((N_CORES, 128, nslot_col), -1.0, np.float32)
    val_all = np.zeros((N_CORES, 128, nslot_col), np.float32)
    lr_all[core_s, p_s, col_s] = lr[order]
    val_all[core_s, p_s, col_s] = vals[order]

    iota = np.broadcast_to(np.arange(128, dtype=np.float32), (128, 128)).copy()
    # per-call valid-index counts, in program call order g -> wl -> q
    cnt = (
        counts.reshape(N_CORES, NGRP, Q, G)
        .transpose(0, 1, 3, 2)
        .reshape(N_CORES, 1, NGRP * G * Q)
        .astype(np.int32)
    )
    return [
        {
            "xg": x,
            "w": weight,
            "idx": idx_all[c],
            "lr": lr_all[c],
            "val": val_all[c],
            "iota": iota,
            "cnt": cnt[c],
        }
        for c in range(N_CORES)
    ]


_PROGRAM = None
_PROGRAM_CPQ = None


def kernel(x, weight, adj_rows, adj_cols, adj_vals):
    global _PROGRAM, _PROGRAM_CPQ
    x = np.ascontiguousarray(np.asarray(x, dtype=np.float32))
    weight = np.ascontiguousarray(np.asarray(weight, dtype=np.float32))
    rows = np.asarray(adj_rows).astype(np.int64)
    cols = np.asarray(adj_cols).astype(np.int64)
    vals = np.asarray(adj_vals).astype(np.float32)

    from concourse.bass_utils import run_bass_kernel_spmd

    cpq = CPQ
    while True:
        try:
            in_maps = shard_inputs(x, weight, rows, cols, vals, cpq=cpq)
            break
        except OverflowError:
            cpq += 1  # statistically ~never; rebuild with more slack
    if _PROGRAM is None or _PROGRAM_CPQ != cpq:
        _PROGRAM = build_program(cpq=cpq, ohbufs=16, oh_eng="vector")
        _PROGRAM_CPQ = cpq
    res = run_bass_kernel_spmd(_PROGRAM, in_maps, list(range(N_CORES))).results

    out = np.empty((N_NODES, F_OUT), np.float32)
    for c in range(N_CORES):
        out[c * R : (c + 1) * R] = res[c]["outT"][:, :R].T
    return out
